# revision 1
# baseline (speedup 1.0000x reference)
"""Trainium2 Bass kernel for nn_ConversationGNN (2-layer GAT, 50K nodes / 500K edges).

Strategy (8 NeuronCores, SPMD, one program):
  - Host: relabel nodes so each core owns 49 windows x 128 nodes, with edges
    (incl. self-loops) bin-packed so every window holds <= F*128 edges. All
    per-core structure (gather indices, one-hot scatter tiles) becomes plain
    input data -> a single uniform program runs on all 8 cores.
  - Device, per core:
      phase A: enc + gat1 linear for own nodes (augmented weights fold the
               attention dot-products a_src/a_dst into extra output columns)
      AllGather the [6272 x 1088] feature+alpha table -> full 50176-row table
      phase C: per edge-tile (128 edges): indirect-DMA gather source rows,
               broadcast dst alpha via one-hot matmul, LeakyReLU+exp on chip,
               softmax-weighted scatter back via one-hot matmul into PSUM
               (numerator and denominator accumulated in one pass), then
               normalize + bias + ELU per 128-dst window; PE-transpose h1 for
               the next layer's matmul.
      phase D: gat2 linear; AllGather table2; phase F: same aggregation for
               layer 2; write output shard.
  - Softmax max-subtraction is skipped: e = leaky(as+ad) is O(+-10) here, so
    exp() is well within f32 range and softmax is shift-invariant.

Self-contained: hardcodes all shapes; only needs numpy + the concourse tree
at /opt/trn_rl_repo (container-provided).
"""

import heapq
import sys

import numpy as np

for _p in ("/opt/trn_rl_repo",):
    if _p not in sys.path:
        sys.path.insert(0, _p)

# problem constants
N = 50000
IN_DIM = 384
HID = 256
HEADS = 4
OUT_DIM = 128
NEG_SLOPE = 0.2

NCORES = 8
P = 128
NW = 49               # dst windows per core
NSH = NW * P          # 6272 padded nodes per core
NT = NCORES * NSH     # 50176 padded nodes total
D1 = HEADS * HID      # 1024
DT1 = 1088            # table1 row: [h1pre(1024) | alpha_src(4) | pad(60)]
DT2 = 192             # table2 row: [h2pre(128) | alpha_src(1) | pad(63)]
W1C = D1 + 8          # 1032: [W1 | a_src_fold(4) | a_dst_fold(4)]
W2C = OUT_DIM + 2     # 130


# ---------------------------------------------------------------- host side

def _pack_nodes(deg):
    """Assign nodes to (window, slot) so window edge-counts are balanced.

    Returns padded ids [N] (window*128 + slot) and per-window edge counts.
    """
    nwg = NCORES * NW
    order = np.argsort(-deg, kind="stable")
    heap = [(0, w) for w in range(nwg)]
    heapq.heapify(heap)
    slots_used = np.zeros(nwg, np.int64)
    edges_w = np.zeros(nwg, np.int64)
    assign_w = np.empty(N, np.int64)
    assign_s = np.empty(N, np.int64)
    for i in order:
        while True:
            _, w = heapq.heappop(heap)
            if slots_used[w] < P:
                break
        assign_w[i] = w
        assign_s[i] = slots_used[w]
        slots_used[w] += 1
        edges_w[w] += deg[i]
        heapq.heappush(heap, (int(edges_w[w]), w))
    return assign_w * P + assign_s, edges_w


def preprocess(x, node_attr, edge_index, enc_W, enc_b,
               W1, a_src1, a_dst1, b1, W2, a_src2, a_dst2, b2):
    x = np.asarray(x, np.float32)
    node_attr = np.asarray(node_attr, np.float32)
    ei = np.asarray(edge_index)
    src_all = np.concatenate([ei[0], np.arange(N, dtype=ei.dtype)]).astype(np.int64)
    dst_all = np.concatenate([ei[1], np.arange(N, dtype=ei.dtype)]).astype(np.int64)
    ne = src_all.shape[0]

    deg = np.bincount(dst_all, minlength=N) + 0  # self-loops already included
    padded, edges_w = _pack_nodes(deg)
    F = int(np.ceil(edges_w.max() / P))
    T = NW * F

    spad = padded[src_all]
    dpad = padded[dst_all]
    wg = dpad // P
    dst_rel = (dpad % P).astype(np.int64)

    order_e = np.argsort(wg, kind="stable")
    wg_s = wg[order_e]
    counts = np.bincount(wg_s, minlength=NCORES * NW)
    starts = np.zeros(NCORES * NW + 1, np.int64)
    starts[1:] = np.cumsum(counts)
    slot = np.arange(ne) - starts[wg_s]

    core_of = wg_s // NW
    w_loc = wg_s % NW
    tile_g = w_loc * F + slot // P
    e_in = slot % P

    SRC = np.zeros((NCORES, T, P), np.int32)
    SED = np.zeros((NCORES, T, P, P), np.float32)
    SRC[core_of, tile_g, e_in] = spad[order_e].astype(np.int32)
    SED[core_of, tile_g, e_in, dst_rel[order_e]] = 1.0

    x2 = np.zeros((NT, 512), np.float32)
    x2[padded, 0:IN_DIM] = x
    x2[padded, IN_DIM:IN_DIM + 2] = node_attr
    x2[padded, IN_DIM + 2] = 1.0

    encWaug = np.zeros((512, IN_DIM), np.float32)
    encWaug[0:IN_DIM + 2] = np.asarray(enc_W, np.float32)
    encWaug[IN_DIM + 2] = np.asarray(enc_b, np.float32)

    W1 = np.asarray(W1, np.float32)
    asrc1t = np.einsum("fhc,hc->fh", W1.reshape(IN_DIM, HEADS, HID),
                       np.asarray(a_src1, np.float32))
    adst1t = np.einsum("fhc,hc->fh", W1.reshape(IN_DIM, HEADS, HID),
                       np.asarray(a_dst1, np.float32))
    W1aug = np.concatenate([W1, asrc1t, adst1t], axis=1)  # [384, 1032]

    W2 = np.asarray(W2, np.float32)
    W2aug = np.concatenate(
        [W2,
         W2 @ np.asarray(a_src2, np.float32)[0][:, None],
         W2 @ np.asarray(a_dst2, np.float32)[0][:, None]], axis=1)  # [1024, 130]

    b1rep = np.tile(np.asarray(b1, np.float32)[None, :], (P, 1))
    b2rep = np.tile(np.asarray(b2, np.float32)[None, :], (P, 1))

    in_maps = []
    for c in range(NCORES):
        in_maps.append({
            "x2T": np.ascontiguousarray(x2[c * NSH:(c + 1) * NSH].T),
            "encW": encWaug,
            "w1aug": W1aug,
            "w2aug": W2aug,
            "b1rep": b1rep,
            "b2rep": b2rep,
            "srcidx": np.ascontiguousarray(SRC[c].T),       # [128, T]
            "s_ed": SED[c],                                  # [T, 128, 128]
            "s_de": np.ascontiguousarray(SED[c].transpose(0, 2, 1)),
        })
    return {"in_maps": in_maps, "F": F, "padded": padded}


# -------------------------------------------------------------- bass program

def build_program(F):
    import concourse.bacc as bacc
    import concourse.bass as bass
    import concourse.mybir as mybir
    import concourse.tile as tile
    from concourse.masks import make_identity

    fp32 = mybir.dt.float32
    i32 = mybir.dt.int32
    Alu = mybir.AluOpType
    Act = mybir.ActivationFunctionType
    T = NW * F

    nc = bacc.Bacc("TRN2", target_bir_lowering=False, debug=False,
                   enable_asserts=False, num_devices=NCORES)

    x2T = nc.dram_tensor("x2T", [512, NSH], fp32, kind="ExternalInput")
    encW = nc.dram_tensor("encW", [512, IN_DIM], fp32, kind="ExternalInput")
    w1aug = nc.dram_tensor("w1aug", [IN_DIM, W1C], fp32, kind="ExternalInput")
    w2aug = nc.dram_tensor("w2aug", [D1, W2C], fp32, kind="ExternalInput")
    b1rep = nc.dram_tensor("b1rep", [P, D1], fp32, kind="ExternalInput")
    b2rep = nc.dram_tensor("b2rep", [P, OUT_DIM], fp32, kind="ExternalInput")
    srcidx = nc.dram_tensor("srcidx", [P, T], i32, kind="ExternalInput")
    s_ed = nc.dram_tensor("s_ed", [T, P, P], fp32, kind="ExternalInput")
    s_de = nc.dram_tensor("s_de", [T, P, P], fp32, kind="ExternalInput")
    out = nc.dram_tensor("out", [NSH, OUT_DIM], fp32, kind="ExternalOutput")

    with tile.TileContext(nc) as tc:
        with (
            tc.tile_pool(name="const", bufs=1) as constp,
            tc.tile_pool(name="dram", bufs=1, space="DRAM") as dram,
        ):
            # ---- persistent SBUF data
            enc_sb = constp.tile([P, 4, IN_DIM], fp32)
            nc.sync.dma_start(enc_sb[:], encW.ap().rearrange("(k p) f -> p k f", p=P))
            w1_sb = constp.tile([P, 3, W1C], fp32)
            nc.sync.dma_start(w1_sb[:], w1aug.ap().rearrange("(k p) f -> p k f", p=P))
            w2_sb = constp.tile([P, 8, W2C], fp32)
            nc.sync.dma_start(w2_sb[:], w2aug.ap().rearrange("(k p) f -> p k f", p=P))
            b1_sb = constp.tile([P, D1], fp32)
            nc.sync.dma_start(b1_sb[:], b1rep.ap())
            b2_sb = constp.tile([P, OUT_DIM], fp32)
            nc.sync.dma_start(b2_sb[:], b2rep.ap())
            sidx_sb = constp.tile([P, T], i32)
            nc.sync.dma_start(sidx_sb[:], srcidx.ap())
            ident = constp.tile([P, P], fp32)
            make_identity(nc, ident[:])
            ad1_sb = constp.tile([P, NW, 4], fp32)
            ad2_sb = constp.tile([P, NW], fp32)

            shard1 = dram.tile([NSH, DT1], fp32)
            table1 = dram.tile([NT, DT1], fp32, addr_space="Shared")
            h1T = dram.tile([D1, NSH], fp32)
            shard2 = dram.tile([NSH, DT2], fp32)
            table2 = dram.tile([NT, DT2], fp32, addr_space="Shared")

            # ================= phase A: enc + gat1 linear =================
            with (
                tc.tile_pool(name="pa_sb", bufs=3) as pa,
                tc.tile_pool(name="pa_ps", bufs=1, space="PSUM") as pap,
                tc.tile_pool(name="pa_ps2", bufs=1, space="PSUM") as pap2,
            ):
                for i in range(NW):
                    xt = pa.tile([P, 4, P], fp32, tag="xt")
                    nc.sync.dma_start(
                        xt[:],
                        x2T.ap().rearrange("(k p) n -> p k n", p=P)[
                            :, :, i * P:(i + 1) * P],
                    )
                    ph0 = pap.tile([P, 3, P], fp32, tag="ph0")
                    for j in range(3):
                        for ks in range(4):
                            nc.tensor.matmul(
                                ph0[:, j, :],
                                lhsT=enc_sb[:, ks, j * P:(j + 1) * P],
                                rhs=xt[:, ks, :],
                                start=(ks == 0), stop=(ks == 3),
                            )
                    h0t = pa.tile([P, 3, P], fp32, tag="h0t")
                    nc.vector.tensor_copy(h0t[:], ph0[:])

                    ph1a = pap2.tile([P, 512], fp32, tag="ph1a")
                    ph1b = pap2.tile([P, 512], fp32, tag="ph1b")
                    ph1c = pap2.tile([P, 8], fp32, tag="ph1c")
                    for ks in range(3):
                        st, sp = (ks == 0), (ks == 2)
                        nc.tensor.matmul(ph1a[:], lhsT=h0t[:, ks, :],
                                         rhs=w1_sb[:, ks, 0:512], start=st, stop=sp)
                        nc.tensor.matmul(ph1b[:], lhsT=h0t[:, ks, :],
                                         rhs=w1_sb[:, ks, 512:1024], start=st, stop=sp)
                        nc.tensor.matmul(ph1c[:], lhsT=h0t[:, ks, :],
                                         rhs=w1_sb[:, ks, 1024:1032], start=st, stop=sp)
                    sh1 = pa.tile([P, DT1], fp32, tag="sh1")
                    nc.vector.tensor_copy(sh1[:, 0:512], ph1a[:])
                    nc.vector.tensor_copy(sh1[:, 512:1024], ph1b[:])
                    nc.vector.tensor_copy(sh1[:, 1024:1028], ph1c[:, 0:4])
                    nc.vector.memset(sh1[:, 1028:DT1], 0.0)
                    nc.vector.tensor_copy(ad1_sb[:, i, :], ph1c[:, 4:8])
                    nc.sync.dma_start(shard1[i * P:(i + 1) * P, :], sh1[:])

            # ================= phase B: all-gather table1 =================
            nc.gpsimd.collective_compute(
                "AllGather", Alu.bypass,
                replica_groups=[list(range(NCORES))],
                ins=[shard1.opt()], outs=[table1.opt()],
            )

            # ================= phase C: gat1 aggregation ==================
            with (
                tc.tile_pool(name="pc_g", bufs=3) as pg,
                tc.tile_pool(name="pc_m", bufs=3) as pm,
                tc.tile_pool(name="pc_s", bufs=4) as psd,
                tc.tile_pool(name="pc_q", bufs=4) as pq,
                tc.tile_pool(name="pc_w", bufs=2) as pw,
                tc.tile_pool(name="pc_po", bufs=2, space="PSUM") as ppo,
                tc.tile_pool(name="pc_sc", bufs=2, space="PSUM") as psc,
            ):
                for w in range(NW):
                    po0 = ppo.tile([P, 512], fp32, tag="po0")
                    po1 = ppo.tile([P, 512], fp32, tag="po1")
                    po2 = ppo.tile([P, 8], fp32, tag="po2")
                    for t in range(F):
                        tg = w * F + t
                        g = pg.tile([P, DT1], fp32, tag="g")
                        nc.gpsimd.indirect_dma_start(
                            out=g[:], out_offset=None, in_=table1[:],
                            in_offset=bass.IndirectOffsetOnAxis(
                                ap=sidx_sb[:, tg:tg + 1], axis=0),
                        )
                        sed = psd.tile([P, P], fp32, tag="sed")
                        nc.sync.dma_start(sed[:], s_ed.ap()[tg])
                        sde = psd.tile([P, P], fp32, tag="sde")
                        nc.sync.dma_start(sde[:], s_de.ap()[tg])

                        pead = psc.tile([P, P], fp32, tag="sc")
                        nc.tensor.matmul(pead[:, 0:4], lhsT=sde[:],
                                         rhs=ad1_sb[:, w, :], start=True, stop=True)
                        q = pq.tile([P, 4], fp32, tag="q")
                        q2 = pq.tile([P, 4], fp32, tag="q2")
                        nc.vector.tensor_add(q[:], g[:, 1024:1028], pead[:, 0:4])
                        nc.vector.tensor_scalar_mul(q2[:], q[:], NEG_SLOPE)
                        nc.vector.tensor_tensor(q[:], q[:], q2[:], op=Alu.max)
                        msg = pm.tile([P, D1 + 4], fp32, tag="msg")
                        nc.scalar.activation(msg[:, D1:D1 + 4], q[:], Act.Exp)
                        nc.vector.tensor_tensor(
                            out=msg[:, 0:D1].rearrange("p (h c) -> p h c", h=HEADS),
                            in0=g[:, 0:D1].rearrange("p (h c) -> p h c", h=HEADS),
                            in1=msg[:, D1:D1 + 4][:, :, None].to_broadcast(
                                [P, HEADS, HID]),
                            op=Alu.mult,
                        )
                        st, sp = (t == 0), (t == F - 1)
                        nc.tensor.matmul(po0[:], lhsT=sed[:], rhs=msg[:, 0:512],
                                         start=st, stop=sp)
                        nc.tensor.matmul(po1[:], lhsT=sed[:], rhs=msg[:, 512:1024],
                                         start=st, stop=sp)
                        nc.tensor.matmul(po2[:, 0:4], lhsT=sed[:],
                                         rhs=msg[:, 1024:1028], start=st, stop=sp)
                    # ---- window drain: softmax-normalize, bias, ELU
                    rden = pq.tile([P, 4], fp32, tag="rden")
                    nc.vector.tensor_scalar_add(rden[:], po2[:, 0:4], 1e-16)
                    nc.vector.reciprocal(rden[:], rden[:])
                    h1 = pw.tile([P, D1], fp32, tag="h1")
                    nc.vector.tensor_tensor(
                        out=h1[:, 0:512].rearrange("p (h c) -> p h c", h=2),
                        in0=po0[:].rearrange("p (h c) -> p h c", h=2),
                        in1=rden[:, 0:2][:, :, None].to_broadcast([P, 2, HID]),
                        op=Alu.mult)
                    nc.vector.tensor_tensor(
                        out=h1[:, 512:1024].rearrange("p (h c) -> p h c", h=2),
                        in0=po1[:].rearrange("p (h c) -> p h c", h=2),
                        in1=rden[:, 2:4][:, :, None].to_broadcast([P, 2, HID]),
                        op=Alu.mult)
                    nc.vector.tensor_add(h1[:], h1[:], b1_sb[:])
                    # ELU(x) = max(x,0) + exp(min(x,0)) - 1
                    em = pw.tile([P, D1], fp32, tag="em")
                    nc.vector.tensor_scalar_min(em[:], h1[:], 0.0)
                    nc.scalar.activation(em[:], em[:], Act.Exp)
                    nc.vector.tensor_scalar_max(h1[:], h1[:], 0.0)
                    nc.vector.tensor_add(h1[:], h1[:], em[:])
                    nc.vector.tensor_scalar_add(h1[:], h1[:], -1.0)
                    # transpose to feature-major for phase D
                    trs = pw.tile([P, 8, P], fp32, tag="trs")
                    for fb in range(8):
                        ptr = psc.tile([P, P], fp32, tag="sc")
                        nc.tensor.transpose(ptr[:], h1[:, fb * P:(fb + 1) * P],
                                            ident[:])
                        nc.vector.tensor_copy(trs[:, fb, :], ptr[:])
                    nc.sync.dma_start(
                        h1T[:].rearrange("(k p) n -> p k n", p=P)[
                            :, :, w * P:(w + 1) * P],
                        trs[:])

            # ================= phase D: gat2 linear =======================
            with (
                tc.tile_pool(name="pd_sb", bufs=3) as pd,
                tc.tile_pool(name="pd_ps", bufs=2, space="PSUM") as pdp,
            ):
                for i in range(NW):
                    ht = pd.tile([P, 8, P], fp32, tag="ht")
                    nc.sync.dma_start(
                        ht[:],
                        h1T[:].rearrange("(k p) n -> p k n", p=P)[
                            :, :, i * P:(i + 1) * P])
                    ph2 = pdp.tile([P, W2C], fp32, tag="ph2")
                    for ks in range(8):
                        nc.tensor.matmul(ph2[:], lhsT=ht[:, ks, :],
                                         rhs=w2_sb[:, ks, :],
                                         start=(ks == 0), stop=(ks == 7))
                    sh2 = pd.tile([P, DT2], fp32, tag="sh2")
                    nc.vector.tensor_copy(sh2[:, 0:OUT_DIM + 1], ph2[:, 0:OUT_DIM + 1])
                    nc.vector.memset(sh2[:, OUT_DIM + 1:DT2], 0.0)
                    nc.vector.tensor_copy(ad2_sb[:, i:i + 1],
                                          ph2[:, OUT_DIM + 1:OUT_DIM + 2])
                    nc.sync.dma_start(shard2[i * P:(i + 1) * P, :], sh2[:])

            # ================= phase E: all-gather table2 =================
            nc.gpsimd.collective_compute(
                "AllGather", Alu.bypass,
                replica_groups=[list(range(NCORES))],
                ins=[shard2.opt()], outs=[table2.opt()],
            )

            # ================= phase F: gat2 aggregation ==================
            with (
                tc.tile_pool(name="pf_g", bufs=4) as pg2,
                tc.tile_pool(name="pf_m", bufs=4) as pm2,
                tc.tile_pool(name="pf_s", bufs=4) as psd2,
                tc.tile_pool(name="pf_q", bufs=4) as pq2,
                tc.tile_pool(name="pf_w", bufs=2) as pw2,
                tc.tile_pool(name="pf_po", bufs=2, space="PSUM") as ppo2,
                tc.tile_pool(name="pf_sc", bufs=2, space="PSUM") as psc2,
            ):
                for w in range(NW):
                    pso = ppo2.tile([P, OUT_DIM + 4], fp32, tag="pso")
                    for t in range(F):
                        tg = w * F + t
                        g = pg2.tile([P, DT2], fp32, tag="g2")
                        nc.gpsimd.indirect_dma_start(
                            out=g[:], out_offset=None, in_=table2[:],
                            in_offset=bass.IndirectOffsetOnAxis(
                                ap=sidx_sb[:, tg:tg + 1], axis=0),
                        )
                        sed = psd2.tile([P, P], fp32, tag="sed2")
                        nc.sync.dma_start(sed[:], s_ed.ap()[tg])
                        sde = psd2.tile([P, P], fp32, tag="sde2")
                        nc.sync.dma_start(sde[:], s_de.ap()[tg])

                        pead = psc2.tile([P, 4], fp32, tag="sc2")
                        nc.tensor.matmul(pead[:, 0:1], lhsT=sde[:],
                                         rhs=ad2_sb[:, w:w + 1], start=True, stop=True)
                        q = pq2.tile([P, 1], fp32, tag="qa")
                        q2 = pq2.tile([P, 1], fp32, tag="qb")
                        nc.vector.tensor_add(q[:], g[:, OUT_DIM:OUT_DIM + 1],
                                             pead[:, 0:1])
                        nc.vector.tensor_scalar_mul(q2[:], q[:], NEG_SLOPE)
                        nc.vector.tensor_tensor(q[:], q[:], q2[:], op=Alu.max)
                        msg = pm2.tile([P, OUT_DIM + 1], fp32, tag="msg2")
                        nc.scalar.activation(msg[:, OUT_DIM:OUT_DIM + 1], q[:],
                                             Act.Exp)
                        nc.vector.tensor_tensor(
                            out=msg[:, 0:OUT_DIM],
                            in0=g[:, 0:OUT_DIM],
                            in1=msg[:, OUT_DIM:OUT_DIM + 1].to_broadcast(
                                [P, OUT_DIM]),
                            op=Alu.mult,
                        )
                        nc.tensor.matmul(pso[:, 0:OUT_DIM + 1], lhsT=sed[:],
                                         rhs=msg[:], start=(t == 0), stop=(t == F - 1))
                    rd2 = pq2.tile([P, 1], fp32, tag="rd2")
                    nc.vector.tensor_scalar_add(rd2[:], pso[:, OUT_DIM:OUT_DIM + 1],
                                                1e-16)
                    nc.vector.reciprocal(rd2[:], rd2[:])
                    ot = pw2.tile([P, OUT_DIM], fp32, tag="ot")
                    nc.vector.tensor_tensor(
                        out=ot[:], in0=pso[:, 0:OUT_DIM],
                        in1=rd2[:].to_broadcast([P, OUT_DIM]), op=Alu.mult)
                    nc.vector.tensor_add(ot[:], ot[:], b2_sb[:])
                    nc.sync.dma_start(out.ap()[w * P:(w + 1) * P, :], ot[:])

    nc.compile()
    return nc


_CACHE = {}


def kernel(**inputs) -> np.ndarray:
    from concourse.bass_utils import run_bass_kernel_spmd

    pre = preprocess(**inputs)
    F = pre["F"]
    if F not in _CACHE:
        _CACHE[F] = build_program(F)
    nc = _CACHE[F]
    res = run_bass_kernel_spmd(nc, pre["in_maps"], core_ids=list(range(NCORES)))
    full = np.concatenate([r["out"] for r in res.results], axis=0)  # [NT, 128]
    return np.ascontiguousarray(full[pre["padded"]]).astype(np.float32)
